# revision 1
# baseline (speedup 1.0000x reference)
"""Trainium2 Bass kernel for ChunkedTropicalAttention.

Shards the fused (batch*head) axis over 8 NeuronCores: core i handles batch
i//4 and heads (2*(i%4), 2*(i%4)+1).  Each core computes t=log1p(relu(x)),
tropical (max-plus) q/k/v projections, the chunked tropical attention, expm1,
and a partial out-projection against its 128-column slice of W_out.  The host
sums the four partials per batch (replicated-W_out head parallelism).

Hot-path dtype is fp16 (DVE 2x mode for the scalar-tensor-tensor max/min
accumulations); accumulation of the final projection is fp32 on the PE.
"""

import os
import sys

sys.path.insert(0, "/opt/trn_rl_repo")

import numpy as np

B, S, DM, NH, DK, CH = 2, 512, 512, 8, 64, 128
NCH = S // CH  # 4 query chunks
HPC = 2        # heads per core
NCORES = 8

_prog = None


def _build_program():
    import concourse.bacc as bacc
    import concourse.mybir as mybir
    from concourse.tile import TileContext

    F32 = mybir.dt.float32
    F16 = mybir.dt.float16
    AF = mybir.ActivationFunctionType
    OP = mybir.AluOpType

    nc = bacc.Bacc("TRN2", target_bir_lowering=False, debug=False,
                   num_devices=NCORES)

    xh = nc.dram_tensor("xh", [S, HPC * DK], F32, kind="ExternalInput")
    wcat = nc.dram_tensor("wcat", [1, DK * 3 * DK], F32, kind="ExternalInput")
    wo = nc.dram_tensor("wo", [HPC * DK, DM], F32, kind="ExternalInput")
    outp = nc.dram_tensor("outp", [S, DM], F32, kind="ExternalOutput")

    NW = DK * 3 * DK  # 12288

    with TileContext(nc) as tc:
        with (
            tc.tile_pool(name="const", bufs=1) as cpool,
            tc.tile_pool(name="tt", bufs=4) as tpool,
            tc.tile_pool(name="acc", bufs=8) as apool,
            tc.tile_pool(name="qf", bufs=8) as qpool,
            tc.tile_pool(name="kvt", bufs=2) as kvtpool,
            tc.tile_pool(name="flat", bufs=2) as fpool,
            tc.tile_pool(name="abA", bufs=2) as aapool,
            tc.tile_pool(name="abB", bufs=2) as bbpool,
            tc.tile_pool(name="sc", bufs=8) as scpool,
            tc.tile_pool(name="scr", bufs=2) as scrpool,
            tc.tile_pool(name="ctx", bufs=4) as ctxpool,
            tc.tile_pool(name="proj", bufs=2) as projpool,
            tc.tile_pool(name="ps", bufs=3, space="PSUM") as pspool,
            tc.tile_pool(name="pso", bufs=2, space="PSUM") as psopool,
        ):
            ones = cpool.tile([1, 128], F16, tag="ones")
            nc.vector.memset(ones[:], 1.0)
            wo_sb = cpool.tile([HPC * DK, DM], F32, tag="wo")
            nc.sync.dma_start(wo_sb[:], wo[:])

            # t = log1p(relu(x)) as 4 fp32 s-tiles [128, 128]
            t_tiles = []
            for st in range(NCH):
                xt_ = tpool.tile([CH, HPC * DK], F32, tag="t")
                nc.sync.dma_start(xt_[:], xh[st * CH:(st + 1) * CH, :])
                nc.vector.tensor_scalar(xt_[:], xt_[:], 0.0, None, OP.max)
                nc.scalar.activation(xt_[:], xt_[:], AF.Ln, bias=1.0, scale=1.0)
                t_tiles.append(xt_)

            # Wb: wcat broadcast across partitions, fp16 [128, 12288]
            qfs = {}
            kvts = {}
            with tc.tile_pool(name="wb", bufs=1) as wbpool:
                wb = wbpool.tile([128, NW], F16, tag="Wb")
                for wch in range(3):
                    wflat = fpool.tile([1, 8 * S], F16, tag="flat")
                    nc.gpsimd.dma_start(
                        wflat[:], wcat[:, wch * 4096:(wch + 1) * 4096])
                    for j in range(8):
                        ps = pspool.tile([128, 512], F32, tag="ps")
                        nc.tensor.matmul(ps[:], ones[:],
                                         wflat[:, j * 512:(j + 1) * 512])
                        nc.scalar.copy(
                            wb[:, wch * 4096 + j * 512: wch * 4096 + (j + 1) * 512],
                            ps[:])

                # tropical linears:
                # acc[h,st][c, w*64+o] = max_i(W_w[o,i] + t[c, h*64+i])
                for h in range(HPC):
                    for st in range(NCH):
                        acc = apool.tile([CH, 3 * DK], F16, tag="acc")
                        for i in range(DK):
                            wbi = wb[:, i * 192:(i + 1) * 192]
                            tcol = t_tiles[st][:, h * DK + i: h * DK + i + 1]
                            if i == 0:
                                nc.vector.tensor_scalar(acc[:], wbi, tcol, None,
                                                        OP.add)
                            else:
                                nc.vector.scalar_tensor_tensor(
                                    acc[:], wbi, tcol, acc[:], OP.add, OP.max)
                        qf = qpool.tile([CH, DK], F32, tag="qf")
                        nc.scalar.copy(qf[:], acc[:, 0:DK])
                        qfs[h, st] = qf
                        if st == 0:
                            kvt_h = kvtpool.tile([128, 512], F16, tag="kvt")
                            kvts[h] = kvt_h
                        nc.sync.dma_start(
                            kvts[h][:, st * CH:(st + 1) * CH],
                            acc[:, DK:3 * DK], transpose=True)

            def build_bcast(h, row0):
                """Broadcast rows [row0, row0+64) of the kvT tile (kT or vT)
                across all 128 partitions -> [128, 64*S] fp16."""
                big = bigpool.tile([128, DK * S], F16, tag="big")
                for j in range(8):
                    flat = fpool.tile([1, 8 * S], F16, tag="flat")
                    nc.sync.dma_start(
                        flat[:], kvts[h][row0 + 8 * j: row0 + 8 * j + 8, :])
                    for half in range(4):
                        d = 8 * j + 2 * half
                        ps = pspool.tile([128, 2 * S], F32, tag="ps")
                        nc.tensor.matmul(ps[:, 0:S], ones[:],
                                         flat[:, 2 * half * S:(2 * half + 1) * S])
                        nc.tensor.matmul(ps[:, S:2 * S], ones[:],
                                         flat[:, (2 * half + 1) * S:(2 * half + 2) * S])
                        nc.scalar.copy(big[:, d * S:(d + 2) * S], ps[:])
                return big

            ctxpairs = []
            for _ch in range(NCH):
                ctxp = ctxpool.tile([CH, HPC * DK], F16, tag="ctxp")
                ctxpairs.append(ctxp)
            scores_tiles = {}
            _bigcm = tc.tile_pool(name="big", bufs=2)
            bigpool = _bigcm.__enter__()
            for h in range(HPC):
                kb = build_bcast(h, 0)      # kT broadcast
                # stage 1: A = max_d(k-q), Bt = min_d(k-q); scores = Bt - A
                for ch in range(NCH):
                    A = aapool.tile([CH, S], F16, tag="A")
                    Bt = bbpool.tile([CH, S], F16, tag="B")
                    qf = qfs[h, ch]
                    nc.vector.tensor_scalar(A[:], kb[:, 0:S], qf[:, 0:1], None,
                                            OP.subtract)
                    nc.vector.tensor_scalar(Bt[:], kb[:, 0:S], qf[:, 0:1], None,
                                            OP.subtract)
                    for d in range(1, DK):
                        kbd = kb[:, d * S:(d + 1) * S]
                        qcol = qf[:, d:d + 1]
                        nc.vector.scalar_tensor_tensor(
                            A[:], kbd, qcol, A[:], OP.subtract, OP.max)
                        nc.vector.scalar_tensor_tensor(
                            Bt[:], kbd, qcol, Bt[:], OP.subtract, OP.min)
                    sc = scpool.tile([CH, S], F16, tag="sc")
                    nc.vector.tensor_tensor(sc[:], Bt[:], A[:], OP.subtract)
                    scores_tiles[h, ch] = sc

                vb = build_bcast(h, DK)     # vT broadcast
                # stage 2: ctx[c, e] = max_s(scores[c,s] + v[s,e])
                # (tensor_tensor_reduce crashes TRN2 here; use TT add +
                #  tensor_reduce max instead)
                for ch in range(NCH):
                    sc = scores_tiles[h, ch]
                    for e in range(DK):
                        scr = scrpool.tile([CH, S], F16, tag="scr")
                        nc.vector.tensor_tensor(
                            scr[:], sc[:], vb[:, e * S:(e + 1) * S], OP.add)
                        nc.vector.tensor_reduce(
                            ctxpairs[ch][:, h * DK + e: h * DK + e + 1],
                            scr[:], axis=mybir.AxisListType.X, op=OP.max)

            _bigcm.__exit__(None, None, None)
            # projection: outp[ch] = (exp(ctx)-1).T-matmul with wo
            for ch in range(NCH):
                eT = projpool.tile([128, 128], F16, tag="eT")
                nc.sync.dma_start(eT[:], ctxpairs[ch][:], transpose=True)
                ex = projpool.tile([128, 128], F32, tag="ex")
                nc.scalar.activation(ex[:], eT[:], AF.Exp)
                nc.vector.tensor_scalar(ex[:], ex[:], -1.0, None, OP.add)
                pso = psopool.tile([128, DM], F32, tag="pso")
                nc.tensor.matmul(pso[:], ex[:], wo_sb[:])
                osb = projpool.tile([128, DM], F32, tag="osb")
                nc.scalar.copy(osb[:], pso[:])
                nc.sync.dma_start(outp[ch * CH:(ch + 1) * CH, :], osb[:])

    nc.compile()
    return nc


def _core_inputs(x, Wq, Wk, Wv, W_out, core):
    b, hp = divmod(core, 4)
    h0 = 2 * hp
    sl = slice(DK * h0, DK * h0 + HPC * DK)
    xh = np.ascontiguousarray(x[b, :, sl], dtype=np.float32)
    wcat = np.ascontiguousarray(
        np.concatenate([Wq.T, Wk.T, Wv.T], axis=1), dtype=np.float32
    ).reshape(1, -1)
    wo = np.ascontiguousarray(W_out[:, sl].T, dtype=np.float32)
    return {"xh": xh, "wcat": wcat, "wo": wo}


_runner = None


def _make_runner(nc):
    """Build the shard_map-jitted executable ONCE (mirrors the multi-core
    path of bass2jax.run_bass_via_pjrt) so repeat calls skip re-tracing."""
    import jax
    import numpy as _np
    from concourse import bass2jax, mybir
    from concourse.bass2jax import (
        Mesh, PartitionSpec, _bass_exec_p, install_neuronx_cc_hook,
        partition_id_tensor, shard_map,
    )

    install_neuronx_cc_hook()
    partition_name = (nc.partition_id_tensor.name
                      if nc.partition_id_tensor else None)
    in_names, out_names, out_avals, zero_outs = [], [], [], []
    for alloc in nc.m.functions[0].allocations:
        if not isinstance(alloc, mybir.MemoryLocationSet):
            continue
        name = alloc.memorylocations[0].name
        if alloc.kind == "ExternalInput":
            if name != partition_name:
                in_names.append(name)
        elif alloc.kind == "ExternalOutput":
            shape = tuple(alloc.tensor_shape)
            dtype = mybir.dt.np(alloc.dtype)
            out_avals.append(jax.core.ShapedArray(shape, dtype))
            out_names.append(name)
            zero_outs.append(_np.zeros(shape, dtype))
    n_params = len(in_names)
    n_outs = len(out_avals)
    all_names = list(in_names) + list(out_names)
    if partition_name is not None:
        all_names.append(partition_name)

    def _body(*args):
        operands = list(args)
        if partition_name is not None:
            operands.append(partition_id_tensor())
        return tuple(_bass_exec_p.bind(
            *operands, out_avals=tuple(out_avals), in_names=tuple(all_names),
            out_names=tuple(out_names), lowering_input_output_aliases=(),
            sim_require_finite=True, sim_require_nnan=True, nc=nc))

    devices = jax.devices()[:NCORES]
    mesh = Mesh(_np.asarray(devices), ("core",))
    in_specs = (PartitionSpec("core"),) * (n_params + n_outs)
    out_specs = (PartitionSpec("core"),) * n_outs
    donate = tuple(range(n_params, n_params + n_outs))
    sharded = jax.jit(
        shard_map(_body, mesh=mesh, in_specs=in_specs, out_specs=out_specs,
                  check_rep=False),
        donate_argnums=donate, keep_unused=True)


    def run(in_maps, fn=None):
        per_core = [[_np.asarray(m[nm]) for nm in in_names] for m in in_maps]
        concat_in = [
            _np.concatenate([per_core[c][i] for c in range(NCORES)], axis=0)
            for i in range(n_params)]
        concat_zeros = [
            _np.zeros((NCORES * z.shape[0], *z.shape[1:]), z.dtype)
            for z in zero_outs]
        out_arrs = (fn or sharded)(*concat_in, *concat_zeros)
        return [
            {nm: _np.asarray(out_arrs[i]).reshape(NCORES, *out_avals[i].shape)[c]
             for i, nm in enumerate(out_names)}
            for c in range(NCORES)]

    return run


def kernel(x, Wq, Wk, Wv, W_out):
    global _prog
    x = np.asarray(x, dtype=np.float32)
    Wq = np.asarray(Wq, dtype=np.float32)
    Wk = np.asarray(Wk, dtype=np.float32)
    Wv = np.asarray(Wv, dtype=np.float32)
    W_out = np.asarray(W_out, dtype=np.float32)

    global _runner
    if _prog is None:
        _prog = _build_program()
    if _runner is None:
        _runner = _make_runner(_prog)

    in_maps = [_core_inputs(x, Wq, Wk, Wv, W_out, c) for c in range(NCORES)]
    results = _runner(in_maps)
    kernel._last = None

    out = np.zeros((B, S, DM), dtype=np.float32)
    for c in range(NCORES):
        out[c // 4] += results[c]["outp"]
    return out


def time_device(x, Wq, Wk, Wv, W_out, n=5):
    """Differential device-time estimate: min over n of t(2 chained execs)
    minus min over n of t(1 exec)."""
    import time as _t
    global _prog, _runner
    if _prog is None:
        _prog = _build_program()
    if _runner is None:
        _runner = _make_runner(_prog)
    in_maps = [_core_inputs(np.asarray(x, np.float32), np.asarray(Wq, np.float32),
                            np.asarray(Wk, np.float32), np.asarray(Wv, np.float32),
                            np.asarray(W_out, np.float32), c)
               for c in range(NCORES)]
    _runner(in_maps)  # warm
    t1 = []
    for _ in range(n):
        t0 = _t.perf_counter()
        _runner(in_maps)
        t1.append(_t.perf_counter() - t0)
    return min(t1) * 1e9, min(t1) * 1e9



# revision 2
# speedup vs baseline: 3.8515x; 3.8515x over previous
"""Trainium2 Bass kernel for ChunkedTropicalAttention.

Shards the fused (batch*head) axis over 8 NeuronCores: core c handles batch
c//4 and heads (2*(c%4), 2*(c%4)+1).  Each core computes t=log1p(relu(x)),
tropical (max-plus) q/k/v projections, the chunked tropical attention, expm1,
and a partial out-projection against its 128-column slice of W_out.  The
partials are summed ON DEVICE with a fp16 ReduceScatter over each batch's
4-core group, so core 4b+r returns only sequence rows [128r, 128(r+1)) of
batch b's final output.

The wall-clock of one call is dominated by the axon tunnel (~70 ms fixed,
~25 ms/MB up, ~31 ms/MB down), so all I/O is fp16 and no donated zero
output buffers are shipped: inputs 280 KB/core up, output 128 KB/core down.
"""

import sys

sys.path.insert(0, "/opt/trn_rl_repo")

import numpy as np

B, S, DM, NH, DK, CH = 2, 512, 512, 8, 64, 128
NCH = S // CH  # 4 query chunks
HPC = 2        # heads per core
NCORES = 8
NW = DK * 3 * DK  # 12288

_prog = None
_runner = None


def _build_program():
    import concourse.bacc as bacc
    import concourse.mybir as mybir
    from concourse.tile import TileContext

    F32 = mybir.dt.float32
    F16 = mybir.dt.float16
    AF = mybir.ActivationFunctionType
    OP = mybir.AluOpType

    nc = bacc.Bacc("TRN2", target_bir_lowering=False, debug=False,
                   num_devices=NCORES)

    xh = nc.dram_tensor("xh", [S, HPC * DK], F16, kind="ExternalInput")
    wcat = nc.dram_tensor("wcat", [1, NW], F16, kind="ExternalInput")
    wo = nc.dram_tensor("wo", [HPC * DK, DM], F16, kind="ExternalInput")
    outp = nc.dram_tensor("outp", [CH, DM], F16, kind="ExternalOutput")

    with TileContext(nc) as tc:
        with (
            tc.tile_pool(name="const", bufs=1) as cpool,
            tc.tile_pool(name="x16", bufs=4) as xpool,
            tc.tile_pool(name="tt", bufs=4) as tpool,
            tc.tile_pool(name="acc", bufs=8) as apool,
            tc.tile_pool(name="qf", bufs=8) as qpool,
            tc.tile_pool(name="kvt", bufs=2) as kvtpool,
            tc.tile_pool(name="flat", bufs=2) as fpool,
            tc.tile_pool(name="abA", bufs=2) as aapool,
            tc.tile_pool(name="abB", bufs=2) as bbpool,
            tc.tile_pool(name="sc", bufs=8) as scpool,
            tc.tile_pool(name="scr", bufs=2) as scrpool,
            tc.tile_pool(name="ctx", bufs=4) as ctxpool,
            tc.tile_pool(name="proj", bufs=2) as projpool,
            tc.tile_pool(name="ps", bufs=3, space="PSUM") as pspool,
            tc.tile_pool(name="pso", bufs=2, space="PSUM") as psopool,
            tc.tile_pool(name="dram", bufs=1, space="DRAM") as dpool,
        ):
            rs_in = dpool.tile([S, DM], F16, tag="rs_in")
            rs_out = dpool.tile([CH, DM], F16, tag="rs_out")

            ones = cpool.tile([1, 128], F16, tag="ones")
            nc.vector.memset(ones[:], 1.0)
            wo_sb = cpool.tile([HPC * DK, DM], F16, tag="wo")
            nc.sync.dma_start(wo_sb[:], wo[:])

            # t = log1p(relu(x)) as 4 fp32 s-tiles [128, 128]
            t_tiles = []
            for st in range(NCH):
                x16 = xpool.tile([CH, HPC * DK], F16, tag="x16")
                nc.sync.dma_start(x16[:], xh[st * CH:(st + 1) * CH, :])
                nc.vector.tensor_scalar(x16[:], x16[:], 0.0, None, OP.max)
                t32 = tpool.tile([CH, HPC * DK], F32, tag="t")
                nc.scalar.activation(t32[:], x16[:], AF.Ln, bias=1.0, scale=1.0)
                t_tiles.append(t32)

            # Wb: wcat broadcast across partitions, fp16 [128, 12288]
            qfs = {}
            kvts = {}
            with tc.tile_pool(name="wb", bufs=1) as wbpool:
                wb = wbpool.tile([128, NW], F16, tag="Wb")
                for wch in range(3):
                    wflat = fpool.tile([1, 8 * S], F16, tag="flat")
                    nc.gpsimd.dma_start(
                        wflat[:], wcat[:, wch * 4096:(wch + 1) * 4096])
                    for j in range(8):
                        ps = pspool.tile([128, 512], F32, tag="ps")
                        nc.tensor.matmul(ps[:], ones[:],
                                         wflat[:, j * 512:(j + 1) * 512])
                        nc.scalar.copy(
                            wb[:, wch * 4096 + j * 512: wch * 4096 + (j + 1) * 512],
                            ps[:])

                # tropical linears:
                # acc[h,st][c, w*64+o] = max_i(W_w[o,i] + t[c, h*64+i])
                for h in range(HPC):
                    for st in range(NCH):
                        acc = apool.tile([CH, 3 * DK], F16, tag="acc")
                        for i in range(DK):
                            wbi = wb[:, i * 192:(i + 1) * 192]
                            tcol = t_tiles[st][:, h * DK + i: h * DK + i + 1]
                            if i == 0:
                                nc.vector.tensor_scalar(acc[:], wbi, tcol, None,
                                                        OP.add)
                            else:
                                nc.vector.scalar_tensor_tensor(
                                    acc[:], wbi, tcol, acc[:], OP.add, OP.max)
                        qf = qpool.tile([CH, DK], F32, tag="qf")
                        nc.scalar.copy(qf[:], acc[:, 0:DK])
                        qfs[h, st] = qf
                        if st == 0:
                            kvt_h = kvtpool.tile([128, 512], F16, tag="kvt")
                            kvts[h] = kvt_h
                        nc.sync.dma_start(
                            kvts[h][:, st * CH:(st + 1) * CH],
                            acc[:, DK:3 * DK], transpose=True)

            def build_bcast(h, row0):
                """Broadcast rows [row0, row0+64) of the kvT tile (kT or vT)
                across all 128 partitions -> [128, 64*S] fp16."""
                big = bigpool.tile([128, DK * S], F16, tag="big")
                for j in range(8):
                    flat = fpool.tile([1, 8 * S], F16, tag="flat")
                    nc.sync.dma_start(
                        flat[:], kvts[h][row0 + 8 * j: row0 + 8 * j + 8, :])
                    for half in range(4):
                        d = 8 * j + 2 * half
                        ps = pspool.tile([128, 2 * S], F32, tag="ps")
                        nc.tensor.matmul(ps[:, 0:S], ones[:],
                                         flat[:, 2 * half * S:(2 * half + 1) * S])
                        nc.tensor.matmul(ps[:, S:2 * S], ones[:],
                                         flat[:, (2 * half + 1) * S:(2 * half + 2) * S])
                        nc.scalar.copy(big[:, d * S:(d + 2) * S], ps[:])
                return big

            ctxpairs = []
            for _ch in range(NCH):
                ctxp = ctxpool.tile([CH, HPC * DK], F16, tag="ctxp")
                ctxpairs.append(ctxp)
            scores_tiles = {}
            _bigcm = tc.tile_pool(name="big", bufs=2)
            bigpool = _bigcm.__enter__()
            for h in range(HPC):
                kb = build_bcast(h, 0)      # kT broadcast
                # stage 1: A = max_d(k-q), Bt = min_d(k-q); scores = Bt - A
                for ch in range(NCH):
                    A = aapool.tile([CH, S], F16, tag="A")
                    Bt = bbpool.tile([CH, S], F16, tag="B")
                    qf = qfs[h, ch]
                    nc.vector.tensor_scalar(A[:], kb[:, 0:S], qf[:, 0:1], None,
                                            OP.subtract)
                    nc.vector.tensor_scalar(Bt[:], kb[:, 0:S], qf[:, 0:1], None,
                                            OP.subtract)
                    for d in range(1, DK):
                        kbd = kb[:, d * S:(d + 1) * S]
                        qcol = qf[:, d:d + 1]
                        nc.vector.scalar_tensor_tensor(
                            A[:], kbd, qcol, A[:], OP.subtract, OP.max)
                        nc.vector.scalar_tensor_tensor(
                            Bt[:], kbd, qcol, Bt[:], OP.subtract, OP.min)
                    sc = scpool.tile([CH, S], F16, tag="sc")
                    nc.vector.tensor_tensor(sc[:], Bt[:], A[:], OP.subtract)
                    scores_tiles[h, ch] = sc

                vb = build_bcast(h, DK)     # vT broadcast
                # stage 2: ctx[c, e] = max_s(scores[c,s] + v[s,e])
                # (tensor_tensor_reduce crashes TRN2 here; use TT add +
                #  tensor_reduce max instead)
                for ch in range(NCH):
                    sc = scores_tiles[h, ch]
                    for e in range(DK):
                        scr = scrpool.tile([CH, S], F16, tag="scr")
                        nc.vector.tensor_tensor(
                            scr[:], sc[:], vb[:, e * S:(e + 1) * S], OP.add)
                        nc.vector.tensor_reduce(
                            ctxpairs[ch][:, h * DK + e: h * DK + e + 1],
                            scr[:], axis=mybir.AxisListType.X, op=OP.max)

            _bigcm.__exit__(None, None, None)
            # projection partial: rs_in[ch] = (exp(ctx)-1) @ wo, fp16
            for ch in range(NCH):
                eT = projpool.tile([128, 128], F16, tag="eT")
                nc.sync.dma_start(eT[:], ctxpairs[ch][:], transpose=True)
                ex = projpool.tile([128, 128], F16, tag="ex")
                nc.scalar.activation(ex[:], eT[:], AF.Exp)
                nc.vector.tensor_scalar(ex[:], ex[:], -1.0, None, OP.add)
                pso = psopool.tile([128, DM], F32, tag="pso")
                nc.tensor.matmul(pso[:], ex[:], wo_sb[:])
                o16 = projpool.tile([128, DM], F16, tag="o16")
                nc.scalar.copy(o16[:], pso[:])
                nc.sync.dma_start(rs_in[ch * CH:(ch + 1) * CH, :], o16[:])

            # on-device partial-sum: fp16 ReduceScatter over each batch's
            # 4-core group; rank r keeps sequence rows [128r, 128(r+1))
            nc.gpsimd.collective_compute(
                "ReduceScatter", OP.add,
                replica_groups=[[0, 1, 2, 3], [4, 5, 6, 7]],
                ins=[rs_in.opt()], outs=[rs_out.opt()])
            nc.sync.dma_start(outp[:], rs_out[:])

    nc.compile()
    return nc


def _core_inputs(x16, wcat16, W_out, core):
    b, hp = divmod(core, 4)
    sl = slice(128 * hp, 128 * hp + 128)
    xh = np.ascontiguousarray(x16[b, :, sl])
    wo = np.ascontiguousarray(W_out[:, sl].T.astype(np.float16))
    return {"xh": xh, "wcat": wcat16, "wo": wo}


def _make_runner(nc):
    """Build the shard_map-jitted executable ONCE. No donated zero output
    buffers (the kernel fully writes outp), fp16 I/O, partition-id appended
    as the last operand (the neuronx_cc_hook expects it)."""
    import jax
    import numpy as _np
    from concourse import mybir
    from concourse.bass2jax import (
        Mesh, PartitionSpec, _bass_exec_p, install_neuronx_cc_hook,
        partition_id_tensor,
    )
    from concourse.bass2jax import shard_map

    install_neuronx_cc_hook()
    partition_name = (nc.partition_id_tensor.name
                      if nc.partition_id_tensor else None)
    in_names, out_names, out_avals = [], [], []
    for alloc in nc.m.functions[0].allocations:
        if not isinstance(alloc, mybir.MemoryLocationSet):
            continue
        name = alloc.memorylocations[0].name
        if alloc.kind == "ExternalInput":
            if name != partition_name:
                in_names.append(name)
        elif alloc.kind == "ExternalOutput":
            shape = tuple(alloc.tensor_shape)
            dtype = mybir.dt.np(alloc.dtype)
            out_avals.append(jax.core.ShapedArray(shape, dtype))
            out_names.append(name)
    n_params = len(in_names)
    all_names = list(in_names)
    if partition_name is not None:
        all_names.append(partition_name)

    def _body(*args):
        operands = list(args)
        if partition_name is not None:
            operands.append(partition_id_tensor())
        return tuple(_bass_exec_p.bind(
            *operands, out_avals=tuple(out_avals), in_names=tuple(all_names),
            out_names=tuple(out_names), lowering_input_output_aliases=(),
            sim_require_finite=True, sim_require_nnan=True, nc=nc))

    devices = jax.devices()[:NCORES]
    mesh = Mesh(_np.asarray(devices), ("core",))
    in_specs = (PartitionSpec("core"),) * n_params
    out_specs = (PartitionSpec("core"),) * len(out_names)
    sharded = jax.jit(
        shard_map(_body, mesh=mesh, in_specs=in_specs, out_specs=out_specs,
                  check_rep=False),
        keep_unused=True)

    def run(in_maps):
        per_core = [[_np.asarray(m[nm]) for nm in in_names] for m in in_maps]
        concat_in = [
            _np.concatenate([per_core[c][i] for c in range(NCORES)], axis=0)
            for i in range(n_params)]
        out_arrs = sharded(*concat_in)
        return [
            {nm: _np.asarray(out_arrs[i]).reshape(NCORES, *out_avals[i].shape)[c]
             for i, nm in enumerate(out_names)}
            for c in range(NCORES)]

    return run


def _prep(x, Wq, Wk, Wv, W_out):
    x16 = np.asarray(x, dtype=np.float16)
    wcat16 = np.ascontiguousarray(
        np.concatenate([np.asarray(Wq).T, np.asarray(Wk).T, np.asarray(Wv).T],
                       axis=1).astype(np.float16)).reshape(1, -1)
    W_out = np.asarray(W_out, dtype=np.float32)
    return [_core_inputs(x16, wcat16, W_out, c) for c in range(NCORES)]


def kernel(x, Wq, Wk, Wv, W_out):
    global _prog, _runner
    if _prog is None:
        _prog = _build_program()
    if _runner is None:
        _runner = _make_runner(_prog)

    in_maps = _prep(x, Wq, Wk, Wv, W_out)
    results = _runner(in_maps)

    out = np.zeros((B, S, DM), dtype=np.float32)
    for c in range(NCORES):
        b, r = divmod(c, 4)
        out[b, 128 * r:128 * (r + 1), :] = results[c]["outp"].astype(np.float32)
    return out


def time_device(x, Wq, Wk, Wv, W_out, n=5):
    """Min wall time of one full device call (includes axon tunnel
    transfers + dispatch)."""
    import time as _t
    global _prog, _runner
    if _prog is None:
        _prog = _build_program()
    if _runner is None:
        _runner = _make_runner(_prog)
    in_maps = _prep(x, Wq, Wk, Wv, W_out)
    _runner(in_maps)  # warm
    t1 = []
    for _ in range(n):
        t0 = _t.perf_counter()
        _runner(in_maps)
        t1.append(_t.perf_counter() - t0)
    return min(t1) * 1e9, min(t1) * 1e9
